# revision 14
# baseline (speedup 1.0000x reference)
"""Trainium2 Bass kernel for the DFS-Mixer style-attention module.

Computation (per batch b):
    dot[k,c]  = sum_hw CT[c,hw] * CR[k,c,hw]
    norm[k,c] = sqrt(sum_hw CR[k,c,hw]^2)
    w[.,c]    = softmax_k(2 * dot[.,c] / norm[.,c])
    out[c,hw] = sum_k IR[k,c,hw] * w[k,c]

Sharding: data-parallel over batch B=8 across the 8 NeuronCores (one b per
core, no cross-core communication).

Per-core layout: C=256 tiled as 2 x 128 SBUF partitions, HW=4096 on the free
axis.  68 MB in / 4 MB out per core -> DMA-roofline kernel (~200 us at
358 GB/s).  Engine budget is kept under the DMA pace in every phase:

- Phase 1 (per CR tile, f32 exact): ACT does ||CR||^2 via Square with
  accumulate; DVE does the dot in a SINGLE fused scalar_tensor_tensor
  ((CT*1)*CR with free-axis sum accum_out).  Per c-tile: ACT ~30us,
  DVE ~35us vs 44.7us of CR DMA -> DMA-bound.
- Softmax over K=8 for both c-tiles back to back (one Sqrt and one Exp
  ACT table switch, hidden under the IR DMA stream).
- Phase 3: IR is cast f32->bf16 in flight by SWDGE (gpsimd) DMA; TensorE
  applies the per-channel weights as bf16 diagonal matmuls accumulating
  in PSUM (full rate, ~15us/c-tile vs 41us for fp32).  ACT (idle here)
  copies PSUM->SBUF per quarter; the output is written as bf16 (half the
  write traffic; quantization well within the error budget).
- Schedule: the CR stream owns the HWDGE sync queue SOLO first (SDMA
  round-robins active queues ~50/50 at packet granularity, so any
  concurrent IR traffic would delay the softmax gate 1:1).  The first
  SWDGE IR cast-DMA is gated on CR tile #10 via a tiny gpsimd read, so
  IR transfers only start overlapping the last few CR tiles; after that
  the IR stream runs on BOTH queues at once -- even k's as f32->bf16
  SWDGE cast-DMAs (gpsimd queue), odd k's as f32 HWDGE loads on the
  now-idle sync queue (reusing CR-pool slots) cast on DVE/ACT.
"""

import os
import sys

import numpy as np


def _import_concourse():
    try:
        import concourse.bass  # noqa: F401
    except ImportError:
        for p in ("/opt/trn_rl_repo", "/root/.axon_site/_ro/trn_rl_repo"):
            if os.path.isdir(p) and p not in sys.path:
                sys.path.insert(0, p)
        import concourse.bass  # noqa: F401


_import_concourse()

import concourse.bass as bass  # noqa: E402
import concourse.mybir as mybir  # noqa: E402
from concourse import tile  # noqa: E402
from concourse.bass_utils import run_bass_kernel_spmd  # noqa: E402
from concourse.vector_clock import ScopedClock, VectorClock  # noqa: E402


def _split_multiwait_bir(bir: bytes) -> bytes:
    """The neuronxcc walrus in this container encodes at most ONE sync-wait
    per instruction; Tile emits several.  Hoist extra waits onto same-engine
    NoOp instructions inserted immediately before the original instruction
    (engines execute in order, so waiting earlier on the same engine is
    semantically identical).  Sem *updates* are left untouched (a DMA's
    completion-inc cannot move to a sequencer NoOp)."""
    import json

    j = json.loads(bir)
    ctr = 0
    for f in j.get("functions", []):
        for bb in f.get("blocks", []):
            out_insts = []
            for ins in bb.get("instructions", []):
                si = ins.get("sync_info")
                waits = (si or {}).get("on_wait") or []
                if len(waits) > 1:
                    for w in waits[:-1]:
                        ctr += 1
                        nop = {
                            "engine": ins["engine"],
                            "ins": [],
                            "outs": [],
                            "name": f"waitsplit-{ctr}",
                            "opcode": "NoOp",
                            "sync_info": {"on_update": [], "on_wait": [w]},
                        }
                        if "debug" in ins:
                            nop["debug"] = ins["debug"]
                        out_insts.append(nop)
                    si["on_wait"] = [waits[-1]]
                out_insts.append(ins)
            bb["instructions"] = out_insts
    return json.dumps(j).encode()


_orig_to_json_bytes = bass.Bass.to_json_bytes


def _patched_to_json_bytes(self, *a, **kw):
    return _split_multiwait_bir(_orig_to_json_bytes(self, *a, **kw))


bass.Bass.to_json_bytes = _patched_to_json_bytes


def _patched_drain_and_barrier(self, tick_clock, wait_clock):
    # Stock TileContext exit emits one Drain waiting on every used semaphore,
    # which this walrus rejects ("Too many sync wait commands").  Emit one
    # Drain per semaphore instead.
    gc = tick_clock.global_clock
    n = len(gc)
    nonzero = [p for p in range(n) if gc[p] > 0] or [0]
    for p in nonzero:
        d = self.nc.sync.drain()
        vec = [gc[q] if q == p else 0 for q in range(n)]
        wait_clock.add_sem_waits(d.ins, ScopedClock({None: VectorClock(vec)}))
    self.nc.all_engine_barrier()
    popped = self.nc._tile_sem_poison_stack.pop()
    assert popped is self._sem_poison
    self.nc.clear_and_free_semaphores(list(self.sems.allocated().values()))
    self.nc.all_engine_barrier()


tile.TileContext._drain_and_barrier = _patched_drain_and_barrier

FP = mybir.dt.float32
BF = mybir.dt.bfloat16
B, K, C, H, W = 8, 8, 256, 64, 64
HW = H * W
P = 128                 # SBUF partitions
NCT = C // P            # 2 c-tiles per core
MMN = 512               # moving free dim per matmul (= one PSUM bank of f32)
NMM = HW // MMN         # 8 matmuls per (k, c-tile)

_AF = mybir.ActivationFunctionType
_OP = mybir.AluOpType
_X = mybir.AxisListType.X


def build_nc() -> bass.Bass:
    nc = bass.Bass()
    IR = nc.declare_dram_parameter("IR", [K, C, HW], FP, isOutput=False)
    CR = nc.declare_dram_parameter("CR", [K, C, HW], FP, isOutput=False)
    CT = nc.declare_dram_parameter("CT", [C, HW], FP, isOutput=False)
    OUT = nc.declare_dram_parameter("OUT", [C, HW], BF, isOutput=True)

    with tile.TileContext(nc) as tc:
        with (
            tc.tile_pool(name="ctp", bufs=1) as ct_pool,
            tc.tile_pool(name="crp", bufs=4) as cr_pool,
            tc.tile_pool(name="irp", bufs=6) as ir_pool,
            tc.tile_pool(name="snk", bufs=1) as sink_pool,
            tc.tile_pool(name="sml", bufs=1) as small,
            tc.tile_pool(name="wkp", bufs=2) as wk_pool,
            tc.tile_pool(name="obp", bufs=2) as out_pool,
            tc.tile_pool(name="psp", bufs=1, space="PSUM") as psum_pool,
        ):
            acc = psum_pool.tile([P, HW], FP, name="acc")
            # Dead elementwise outputs of the phase-1 reductions; one sink
            # per engine so ACT and DVE never serialize on a WAW hazard.
            asink = sink_pool.tile([P, HW], BF, name="asink", tag="asink")
            vsink = sink_pool.tile([P, HW], BF, name="vsink", tag="vsink")

            # Diagonal ones mask, built once: mask[p, f] = (p == f).
            ones_t = small.tile([P, P], FP, name="ones_t")
            nc.vector.memset(ones_t[:], 1.0)
            mask = small.tile([P, P], FP, name="mask")
            nc.gpsimd.affine_select(
                mask[:],
                ones_t[:],
                pattern=[[-1, P]],
                compare_op=_OP.is_equal,
                fill=0.0,
                base=0,
                channel_multiplier=1,
            )

            # Content-target features stay resident in SBUF (reused by all k).
            ct_tiles = []
            for t in range(NCT):
                ctt = ct_pool.tile([P, HW], FP, name=f"ct{t}", tag=f"ct{t}")
                nc.sync.dma_start(out=ctt[:], in_=CT[t * P:(t + 1) * P, :])
                ct_tiles.append(ctt)

            # ---- Phase 1: dot[c,k] and sq[c,k] reductions over HW ----
            # ACT: ||CR||^2 via Square+accumulate.  DVE: dot via one fused
            # tensor_tensor_reduce (CT*CR, free-axis sum).  Both engines run
            # below the CR DMA pace, so this phase streams at HBM rate.
            dots = []
            sqs = []
            cr_gate = None
            for t in range(NCT):
                cs = slice(t * P, (t + 1) * P)
                dot = small.tile([P, K], FP, name=f"dot{t}", tag=f"dot{t}")
                sq = small.tile([P, K], FP, name=f"sq{t}", tag=f"sq{t}")
                for k in range(K):
                    crt = cr_pool.tile([P, HW], FP, name="crt", tag="cr")
                    nc.sync.dma_start(out=crt[:], in_=CR[k, cs, :])
                    if t * K + k == 11:  # CR tile #12 -> the IR-stream gate
                        cr_gate = crt
                    nc.scalar.activation(
                        out=asink[:], in_=crt[:], func=_AF.Square,
                        accum_out=sq[:, k:k + 1],
                    )
                    nc.vector.scalar_tensor_tensor(
                        out=vsink[:],
                        in0=ct_tiles[t][:],
                        scalar=1.0,
                        in1=crt[:],
                        op0=_OP.mult,
                        op1=_OP.mult,
                        accum_out=dot[:, k:k + 1],
                    )
                dots.append(dot)
                sqs.append(sq)

            # ---- Softmax weights (tiny [128, K] ops) ----
            # Both sqrts first, then both exps: exactly one Sqrt and one Exp
            # ACT table switch, overlapped with the IR DMA stream.
            norms = []
            for t in range(NCT):
                norm = small.tile([P, K], FP, name=f"norm{t}", tag=f"norm{t}")
                nc.scalar.sqrt(norm[:], sqs[t][:])
                norms.append(norm)
            ws = []
            for t in range(NCT):
                rnorm = small.tile([P, K], FP, name=f"rnorm{t}", tag=f"rnorm{t}")
                nc.vector.reciprocal(rnorm[:], norms[t][:])
                sim = small.tile([P, K], FP, name=f"sim{t}", tag=f"sim{t}")
                nc.vector.tensor_mul(sim[:], dots[t][:], rnorm[:])
                mx = small.tile([P, 1], FP, name=f"mx{t}", tag=f"mx{t}")
                nc.vector.reduce_max(mx[:], sim[:], axis=_X)
                nbias = small.tile([P, 1], FP, name=f"nb{t}", tag=f"nb{t}")
                nc.vector.tensor_scalar_mul(nbias[:], mx[:], -2.0)
                e = small.tile([P, K], FP, name=f"e{t}", tag=f"e{t}")
                nc.scalar.activation(
                    e[:], sim[:], func=_AF.Exp, bias=nbias[:, 0:1], scale=2.0
                )
                s = small.tile([P, 1], FP, name=f"s{t}", tag=f"s{t}")
                nc.vector.reduce_sum(s[:], e[:], axis=_X)
                rs = small.tile([P, 1], FP, name=f"rs{t}", tag=f"rs{t}")
                nc.vector.reciprocal(rs[:], s[:])
                w = small.tile([P, K], FP, name=f"w{t}", tag=f"w{t}")
                nc.vector.tensor_scalar_mul(w[:], e[:], rs[:, 0:1])
                ws.append(w)

            # ---- Phase 3: out[c,:] = sum_k IR[k,c,:] * w[c,k] ----
            # IR is cast f32->bf16 in flight (SWDGE), TensorE applies the
            # per-channel weight as a bf16 diagonal matmul into PSUM.
            QQ = HW // 4
            for t in range(NCT):
                cs = slice(t * P, (t + 1) * P)
                w = ws[t]
                # Process odd (sync-queue f32) and even (SWDGE cast) k's
                # interleaved so the final tiles of each c-tile arrive on
                # alternating queues and the tail never leaves one queue
                # idle.  PSUM accumulation order over k is irrelevant.
                korder = [1, 0, 3, 2, 5, 4, 7, 6]
                for ki, k in enumerate(korder):
                    wm = wk_pool.tile([P, P], BF, name="wm", tag="wm")
                    nc.vector.tensor_scalar_mul(wm[:], mask[:], w[:, k:k + 1])
                    irt = ir_pool.tile([P, HW], BF, name="irt", tag="ir")
                    if k % 2 == 0:
                        if t == 0 and k == 0:
                            # Gate: scribble a CR-#10-derived value into the
                            # first IR tile so this SWDGE DMA (WAW on the
                            # tile) cannot start before CR #10 has landed;
                            # Q7 ring backpressure paces the rest.
                            nc.gpsimd.tensor_copy(
                                irt[0:1, 0:1], cr_gate[0:1, 0:1]
                            )
                        nc.gpsimd.dma_start(out=irt[:], in_=IR[k, cs, :])
                    else:
                        # Odd k: f32 on the post-CR-idle sync queue, reusing
                        # CR-pool slots (whose WARs naturally pace them to
                        # the CR tail), cast to bf16 on DVE/ACT.
                        irf = cr_pool.tile([P, HW], FP, name="irf", tag="cr")
                        nc.sync.dma_start(out=irf[:], in_=IR[k, cs, :])
                        if (k // 2) % 2 == 0:
                            nc.vector.tensor_copy(irt[:], irf[:])
                        else:
                            nc.scalar.copy(irt[:], irf[:])
                    for j in range(NMM):
                        col = j * MMN
                        nc.tensor.matmul(
                            acc[:, col:col + MMN],
                            wm[:],
                            irt[:, col:col + MMN],
                            start=(ki == 0),
                            stop=(ki == K - 1),
                        )
                # PSUM -> SBUF on ACT (idle here), stream each quarter out
                # as soon as its last matmul lands.
                ob = out_pool.tile([P, HW], BF, name="ob", tag="ob")
                for q in range(4):
                    qs = slice(q * QQ, (q + 1) * QQ)
                    nc.scalar.copy(ob[:, qs], acc[:, qs])
                    nc.sync.dma_start(out=OUT[cs, qs], in_=ob[:, qs])

    return nc


_NC_CACHE = None


def _get_nc() -> bass.Bass:
    global _NC_CACHE
    if _NC_CACHE is None:
        _NC_CACHE = build_nc()
    return _NC_CACHE


def run(inputs: dict, trace: bool = False):
    """Shard over B, run on 8 cores, gather. Returns (output, BassKernelResults)."""
    ir = np.ascontiguousarray(np.asarray(inputs["IR_features"], dtype=np.float32))
    cr = np.ascontiguousarray(np.asarray(inputs["CR_features"], dtype=np.float32))
    ct = np.ascontiguousarray(np.asarray(inputs["CT_feature"], dtype=np.float32))
    assert ir.shape == (B, K, C, H, W) and cr.shape == (B, K, C, H, W)
    assert ct.shape == (B, C, H, W)

    in_maps = [
        {
            "IR": ir[b].reshape(K, C, HW),
            "CR": cr[b].reshape(K, C, HW),
            "CT": ct[b].reshape(C, HW),
        }
        for b in range(B)
    ]
    res = run_bass_kernel_spmd(_get_nc(), in_maps, list(range(B)), trace=trace)
    out = np.stack([res.results[b]["OUT"] for b in range(B)])
    return out.reshape(B, C, H, W).astype(np.float32), res


def kernel(**inputs) -> np.ndarray:
    return run(inputs)[0]


# revision 19
# speedup vs baseline: 1.0113x; 1.0113x over previous
"""Trainium2 Bass kernel for the DFS-Mixer style-attention module.

Computation (per batch b):
    dot[k,c]  = sum_hw CT[c,hw] * CR[k,c,hw]
    norm[k,c] = sqrt(sum_hw CR[k,c,hw]^2)
    w[.,c]    = softmax_k(2 * dot[.,c] / norm[.,c])
    out[c,hw] = sum_k IR[k,c,hw] * w[k,c]

Sharding: data-parallel over batch B=8 across the 8 NeuronCores (one b per
core, no cross-core communication).

Per-core layout: C=256 tiled as 2 x 128 SBUF partitions, HW=4096 on the free
axis.  68 MB in / 4 MB out per core -> DMA-roofline kernel (~200 us at
358 GB/s).  Engine budget is kept under the DMA pace in every phase:

- Phase 1 (per CR tile, f32 exact): ACT does ||CR||^2 via Square with
  accumulate; DVE does the dot in a SINGLE fused scalar_tensor_tensor
  ((CT*1)*CR with free-axis sum accum_out).  Per c-tile: ACT ~30us,
  DVE ~35us vs 44.7us of CR DMA -> DMA-bound.
- Softmax over K=8 for both c-tiles back to back (one Sqrt and one Exp
  ACT table switch, hidden under the IR DMA stream).
- Phase 3: IR is cast f32->bf16 in flight by SWDGE (gpsimd) DMA; TensorE
  applies the per-channel weights as bf16 diagonal matmuls accumulating
  in PSUM (full rate, ~15us/c-tile vs 41us for fp32).  ACT (idle here)
  copies PSUM->SBUF per quarter; the output is written as bf16 (half the
  write traffic; quantization well within the error budget).
- Schedule: the CR stream owns the HWDGE sync queue SOLO first (SDMA
  round-robins active queues ~50/50 at packet granularity, so any
  concurrent IR traffic would delay the softmax gate 1:1).  The first
  SWDGE IR cast-DMA is gated on CR tile #10 via a tiny gpsimd read, so
  IR transfers only start overlapping the last few CR tiles; after that
  the IR stream runs on BOTH queues at once -- even k's as f32->bf16
  SWDGE cast-DMAs (gpsimd queue), odd k's as f32 HWDGE loads on the
  now-idle sync queue (reusing CR-pool slots) cast on DVE/ACT.
"""

import os
import sys

import numpy as np


def _import_concourse():
    try:
        import concourse.bass  # noqa: F401
    except ImportError:
        for p in ("/opt/trn_rl_repo", "/root/.axon_site/_ro/trn_rl_repo"):
            if os.path.isdir(p) and p not in sys.path:
                sys.path.insert(0, p)
        import concourse.bass  # noqa: F401


_import_concourse()

import concourse.bass as bass  # noqa: E402
import concourse.mybir as mybir  # noqa: E402
from concourse import tile  # noqa: E402
from concourse.bass_utils import run_bass_kernel_spmd  # noqa: E402
from concourse.vector_clock import ScopedClock, VectorClock  # noqa: E402


def _split_multiwait_bir(bir: bytes) -> bytes:
    """The neuronxcc walrus in this container encodes at most ONE sync-wait
    per instruction; Tile emits several.  Hoist extra waits onto same-engine
    NoOp instructions inserted immediately before the original instruction
    (engines execute in order, so waiting earlier on the same engine is
    semantically identical).  Sem *updates* are left untouched (a DMA's
    completion-inc cannot move to a sequencer NoOp)."""
    import json

    j = json.loads(bir)
    ctr = 0
    for f in j.get("functions", []):
        for bb in f.get("blocks", []):
            out_insts = []
            for ins in bb.get("instructions", []):
                si = ins.get("sync_info")
                waits = (si or {}).get("on_wait") or []
                if len(waits) > 1:
                    for w in waits[:-1]:
                        ctr += 1
                        nop = {
                            "engine": ins["engine"],
                            "ins": [],
                            "outs": [],
                            "name": f"waitsplit-{ctr}",
                            "opcode": "NoOp",
                            "sync_info": {"on_update": [], "on_wait": [w]},
                        }
                        if "debug" in ins:
                            nop["debug"] = ins["debug"]
                        out_insts.append(nop)
                    si["on_wait"] = [waits[-1]]
                out_insts.append(ins)
            bb["instructions"] = out_insts
    return json.dumps(j).encode()


_orig_to_json_bytes = bass.Bass.to_json_bytes


def _patched_to_json_bytes(self, *a, **kw):
    return _split_multiwait_bir(_orig_to_json_bytes(self, *a, **kw))


bass.Bass.to_json_bytes = _patched_to_json_bytes


def _patched_drain_and_barrier(self, tick_clock, wait_clock):
    # Stock TileContext exit emits one Drain waiting on every used semaphore,
    # which this walrus rejects ("Too many sync wait commands").  Emit one
    # Drain per semaphore instead.
    gc = tick_clock.global_clock
    n = len(gc)
    nonzero = [p for p in range(n) if gc[p] > 0] or [0]
    for p in nonzero:
        d = self.nc.sync.drain()
        vec = [gc[q] if q == p else 0 for q in range(n)]
        wait_clock.add_sem_waits(d.ins, ScopedClock({None: VectorClock(vec)}))
    self.nc.all_engine_barrier()
    popped = self.nc._tile_sem_poison_stack.pop()
    assert popped is self._sem_poison
    self.nc.clear_and_free_semaphores(list(self.sems.allocated().values()))
    self.nc.all_engine_barrier()


tile.TileContext._drain_and_barrier = _patched_drain_and_barrier

FP = mybir.dt.float32
BF = mybir.dt.bfloat16
B, K, C, H, W = 8, 8, 256, 64, 64
HW = H * W
P = 128                 # SBUF partitions
NCT = C // P            # 2 c-tiles per core
MMN = 512               # moving free dim per matmul (= one PSUM bank of f32)
NMM = HW // MMN         # 8 matmuls per (k, c-tile)

_AF = mybir.ActivationFunctionType
_OP = mybir.AluOpType
_X = mybir.AxisListType.X


def build_nc() -> bass.Bass:
    nc = bass.Bass()
    IR = nc.declare_dram_parameter("IR", [K, C, HW], FP, isOutput=False)
    CR = nc.declare_dram_parameter("CR", [K, C, HW], FP, isOutput=False)
    CT = nc.declare_dram_parameter("CT", [C, HW], FP, isOutput=False)
    OUT = nc.declare_dram_parameter("OUT", [C, HW], BF, isOutput=True)

    with tile.TileContext(nc) as tc:
        with (
            tc.tile_pool(name="ctp", bufs=1) as ct_pool,
            tc.tile_pool(name="crp", bufs=4) as cr_pool,
            tc.tile_pool(name="irp", bufs=7) as ir_pool,
            tc.tile_pool(name="snk", bufs=1) as sink_pool,
            tc.tile_pool(name="sml", bufs=1) as small,
            tc.tile_pool(name="wkp", bufs=2) as wk_pool,
            tc.tile_pool(name="obp", bufs=2) as out_pool,
            tc.tile_pool(name="psp", bufs=1, space="PSUM") as psum_pool,
        ):
            acc = psum_pool.tile([P, HW], FP, name="acc")
            # Dead elementwise outputs of the phase-1 reductions; one sink
            # per engine so ACT and DVE never serialize on a WAW hazard.
            asink = sink_pool.tile([P, HW], BF, name="asink", tag="asink")
            vsink = sink_pool.tile([P, HW], BF, name="vsink", tag="vsink")

            # Diagonal ones mask, built once: mask[p, f] = (p == f).
            ones_t = small.tile([P, P], FP, name="ones_t")
            nc.vector.memset(ones_t[:], 1.0)
            mask = small.tile([P, P], FP, name="mask")
            nc.gpsimd.affine_select(
                mask[:],
                ones_t[:],
                pattern=[[-1, P]],
                compare_op=_OP.is_equal,
                fill=0.0,
                base=0,
                channel_multiplier=1,
            )

            # Content-target features stay resident in SBUF (reused by all k).
            ct_tiles = []
            for t in range(NCT):
                ctt = ct_pool.tile([P, HW], FP, name=f"ct{t}", tag=f"ct{t}")
                nc.sync.dma_start(out=ctt[:], in_=CT[t * P:(t + 1) * P, :])
                ct_tiles.append(ctt)

            # ---- Phase 1: dot[c,k] and sq[c,k] reductions over HW ----
            # ACT: ||CR||^2 via Square+accumulate.  DVE: dot via one fused
            # tensor_tensor_reduce (CT*CR, free-axis sum).  Both engines run
            # below the CR DMA pace, so this phase streams at HBM rate.
            dots = []
            sqs = []
            cr_gate = None
            for t in range(NCT):
                cs = slice(t * P, (t + 1) * P)
                dot = small.tile([P, K], FP, name=f"dot{t}", tag=f"dot{t}")
                sq = small.tile([P, K], FP, name=f"sq{t}", tag=f"sq{t}")
                for k in range(K):
                    crt = cr_pool.tile([P, HW], FP, name="crt", tag="cr")
                    nc.sync.dma_start(out=crt[:], in_=CR[k, cs, :])
                    if t * K + k == 9:  # CR tile #10 -> the IR-stream gate
                        cr_gate = crt
                    nc.scalar.activation(
                        out=asink[:], in_=crt[:], func=_AF.Square,
                        accum_out=sq[:, k:k + 1],
                    )
                    nc.vector.scalar_tensor_tensor(
                        out=vsink[:],
                        in0=ct_tiles[t][:],
                        scalar=1.0,
                        in1=crt[:],
                        op0=_OP.mult,
                        op1=_OP.mult,
                        accum_out=dot[:, k:k + 1],
                    )
                dots.append(dot)
                sqs.append(sq)

            # ---- Softmax weights (tiny [128, K] ops) ----
            # Both sqrts first, then both exps: exactly one Sqrt and one Exp
            # ACT table switch, overlapped with the IR DMA stream.
            norms = []
            for t in range(NCT):
                norm = small.tile([P, K], FP, name=f"norm{t}", tag=f"norm{t}")
                nc.scalar.sqrt(norm[:], sqs[t][:])
                norms.append(norm)
            ws = []
            for t in range(NCT):
                rnorm = small.tile([P, K], FP, name=f"rnorm{t}", tag=f"rnorm{t}")
                nc.vector.reciprocal(rnorm[:], norms[t][:])
                sim = small.tile([P, K], FP, name=f"sim{t}", tag=f"sim{t}")
                nc.vector.tensor_mul(sim[:], dots[t][:], rnorm[:])
                mx = small.tile([P, 1], FP, name=f"mx{t}", tag=f"mx{t}")
                nc.vector.reduce_max(mx[:], sim[:], axis=_X)
                nbias = small.tile([P, 1], FP, name=f"nb{t}", tag=f"nb{t}")
                nc.vector.tensor_scalar_mul(nbias[:], mx[:], -2.0)
                e = small.tile([P, K], FP, name=f"e{t}", tag=f"e{t}")
                nc.scalar.activation(
                    e[:], sim[:], func=_AF.Exp, bias=nbias[:, 0:1], scale=2.0
                )
                s = small.tile([P, 1], FP, name=f"s{t}", tag=f"s{t}")
                nc.vector.reduce_sum(s[:], e[:], axis=_X)
                rs = small.tile([P, 1], FP, name=f"rs{t}", tag=f"rs{t}")
                nc.vector.reciprocal(rs[:], s[:])
                w = small.tile([P, K], FP, name=f"w{t}", tag=f"w{t}")
                nc.vector.tensor_scalar_mul(w[:], e[:], rs[:, 0:1])
                ws.append(w)

            # ---- Phase 3: out[c,:] = sum_k IR[k,c,:] * w[c,k] ----
            # IR is cast f32->bf16 in flight (SWDGE), TensorE applies the
            # per-channel weight as a bf16 diagonal matmul into PSUM.
            QQ = HW // 4
            for t in range(NCT):
                cs = slice(t * P, (t + 1) * P)
                w = ws[t]
                for ki, k in enumerate(range(K)):
                    wm = wk_pool.tile([P, P], BF, name="wm", tag="wm")
                    nc.vector.tensor_scalar_mul(wm[:], mask[:], w[:, k:k + 1])
                    irt = ir_pool.tile([P, HW], BF, name="irt", tag="ir")
                    if k % 2 == 0:
                        if t == 0 and k == 0:
                            # Gate: scribble a CR-#10-derived value into the
                            # first IR tile so this SWDGE DMA (WAW on the
                            # tile) cannot start before CR #10 has landed;
                            # Q7 ring backpressure paces the rest.
                            nc.gpsimd.tensor_copy(
                                irt[0:1, 0:1], cr_gate[0:1, 0:1]
                            )
                        nc.gpsimd.dma_start(out=irt[:], in_=IR[k, cs, :])
                    else:
                        # Odd k: f32 on the post-CR-idle sync queue, reusing
                        # CR-pool slots (whose WARs naturally pace them to
                        # the CR tail), cast to bf16 on DVE/ACT.
                        irf = cr_pool.tile([P, HW], FP, name="irf", tag="cr")
                        nc.sync.dma_start(out=irf[:], in_=IR[k, cs, :])
                        if (k // 2) % 2 == 0:
                            nc.scalar.copy(irt[:], irf[:])
                        else:
                            # k=5,7 on DVE: keeps the last tile's cast off
                            # ACT, which still owes the PSUM quarter copies.
                            nc.vector.tensor_copy(irt[:], irf[:])
                    for j in range(NMM):
                        col = j * MMN
                        nc.tensor.matmul(
                            acc[:, col:col + MMN],
                            wm[:],
                            irt[:, col:col + MMN],
                            start=(ki == 0),
                            stop=(ki == K - 1),
                        )
                # PSUM -> SBUF on ACT (idle here), stream each quarter out
                # as soon as its last matmul lands.
                ob = out_pool.tile([P, HW], BF, name="ob", tag="ob")
                for q in range(4):
                    qs = slice(q * QQ, (q + 1) * QQ)
                    nc.scalar.copy(ob[:, qs], acc[:, qs])
                    # Scalar-queue (third HWDGE ring): output writes never
                    # contend with the sync queue's remaining IR loads.
                    nc.scalar.dma_start(out=OUT[cs, qs], in_=ob[:, qs])

    return nc


_NC_CACHE = None


def _get_nc() -> bass.Bass:
    global _NC_CACHE
    if _NC_CACHE is None:
        _NC_CACHE = build_nc()
    return _NC_CACHE


def run(inputs: dict, trace: bool = False):
    """Shard over B, run on 8 cores, gather. Returns (output, BassKernelResults)."""
    ir = np.ascontiguousarray(np.asarray(inputs["IR_features"], dtype=np.float32))
    cr = np.ascontiguousarray(np.asarray(inputs["CR_features"], dtype=np.float32))
    ct = np.ascontiguousarray(np.asarray(inputs["CT_feature"], dtype=np.float32))
    assert ir.shape == (B, K, C, H, W) and cr.shape == (B, K, C, H, W)
    assert ct.shape == (B, C, H, W)

    in_maps = [
        {
            "IR": ir[b].reshape(K, C, HW),
            "CR": cr[b].reshape(K, C, HW),
            "CT": ct[b].reshape(C, HW),
        }
        for b in range(B)
    ]
    res = run_bass_kernel_spmd(_get_nc(), in_maps, list(range(B)), trace=trace)
    out = np.stack([res.results[b]["OUT"] for b in range(B)])
    return out.reshape(B, C, H, W).astype(np.float32), res


def kernel(**inputs) -> np.ndarray:
    return run(inputs)[0]


# revision 21
# speedup vs baseline: 1.0160x; 1.0046x over previous
"""Trainium2 Bass kernel for the DFS-Mixer style-attention module.

Computation (per batch b):
    dot[k,c]  = sum_hw CT[c,hw] * CR[k,c,hw]
    norm[k,c] = sqrt(sum_hw CR[k,c,hw]^2)
    w[.,c]    = softmax_k(2 * dot[.,c] / norm[.,c])
    out[c,hw] = sum_k IR[k,c,hw] * w[k,c]

Sharding: data-parallel over batch B=8 across the 8 NeuronCores (one b per
core, no cross-core communication).

Per-core layout: C=256 tiled as 2 x 128 SBUF partitions, HW=4096 on the free
axis.  68 MB in / 4 MB out per core -> DMA-roofline kernel (~200 us at
358 GB/s).  Engine budget is kept under the DMA pace in every phase:

- Phase 1 (per CR tile, f32 exact): ACT does ||CR||^2 via Square with
  accumulate; DVE does the dot in a SINGLE fused scalar_tensor_tensor
  ((CT*1)*CR with free-axis sum accum_out).  Per c-tile: ACT ~30us,
  DVE ~35us vs 44.7us of CR DMA -> DMA-bound.
- Softmax over K=8 for both c-tiles back to back (one Sqrt and one Exp
  ACT table switch, hidden under the IR DMA stream).
- Phase 3: IR is cast f32->bf16 in flight by SWDGE (gpsimd) DMA; TensorE
  applies the per-channel weights as bf16 diagonal matmuls accumulating
  in PSUM (full rate, ~15us/c-tile vs 41us for fp32).  ACT (idle here)
  copies PSUM->SBUF per quarter; the output is written as bf16 (half the
  write traffic; quantization well within the error budget).
- Schedule: the CR stream owns the HWDGE sync queue SOLO first (SDMA
  round-robins active queues ~50/50 at packet granularity, so any
  concurrent IR traffic would delay the softmax gate 1:1).  The first
  SWDGE IR cast-DMA is gated on CR tile #10 via a tiny gpsimd read, so
  IR transfers only start overlapping the last few CR tiles; after that
  the IR stream runs on BOTH queues at once -- even k's as f32->bf16
  SWDGE cast-DMAs (gpsimd queue), odd k's as f32 HWDGE loads on the
  now-idle sync queue (reusing CR-pool slots) cast on DVE/ACT.
"""

import os
import sys

import numpy as np


def _import_concourse():
    try:
        import concourse.bass  # noqa: F401
    except ImportError:
        for p in ("/opt/trn_rl_repo", "/root/.axon_site/_ro/trn_rl_repo"):
            if os.path.isdir(p) and p not in sys.path:
                sys.path.insert(0, p)
        import concourse.bass  # noqa: F401


_import_concourse()

import concourse.bass as bass  # noqa: E402
import concourse.mybir as mybir  # noqa: E402
from concourse import tile  # noqa: E402
from concourse.bass_utils import run_bass_kernel_spmd  # noqa: E402
from concourse.vector_clock import ScopedClock, VectorClock  # noqa: E402


def _split_multiwait_bir(bir: bytes) -> bytes:
    """The neuronxcc walrus in this container encodes at most ONE sync-wait
    per instruction; Tile emits several.  Hoist extra waits onto same-engine
    NoOp instructions inserted immediately before the original instruction
    (engines execute in order, so waiting earlier on the same engine is
    semantically identical).  Sem *updates* are left untouched (a DMA's
    completion-inc cannot move to a sequencer NoOp)."""
    import json

    j = json.loads(bir)
    ctr = 0
    for f in j.get("functions", []):
        for bb in f.get("blocks", []):
            out_insts = []
            for ins in bb.get("instructions", []):
                si = ins.get("sync_info")
                waits = (si or {}).get("on_wait") or []
                if len(waits) > 1:
                    for w in waits[:-1]:
                        ctr += 1
                        nop = {
                            "engine": ins["engine"],
                            "ins": [],
                            "outs": [],
                            "name": f"waitsplit-{ctr}",
                            "opcode": "NoOp",
                            "sync_info": {"on_update": [], "on_wait": [w]},
                        }
                        if "debug" in ins:
                            nop["debug"] = ins["debug"]
                        out_insts.append(nop)
                    si["on_wait"] = [waits[-1]]
                out_insts.append(ins)
            bb["instructions"] = out_insts
    return json.dumps(j).encode()


_orig_to_json_bytes = bass.Bass.to_json_bytes


def _patched_to_json_bytes(self, *a, **kw):
    return _split_multiwait_bir(_orig_to_json_bytes(self, *a, **kw))


bass.Bass.to_json_bytes = _patched_to_json_bytes


def _patched_drain_and_barrier(self, tick_clock, wait_clock):
    # Stock TileContext exit emits one Drain waiting on every used semaphore,
    # which this walrus rejects ("Too many sync wait commands").  Emit one
    # Drain per semaphore instead.
    gc = tick_clock.global_clock
    n = len(gc)
    nonzero = [p for p in range(n) if gc[p] > 0] or [0]
    for p in nonzero:
        d = self.nc.sync.drain()
        vec = [gc[q] if q == p else 0 for q in range(n)]
        wait_clock.add_sem_waits(d.ins, ScopedClock({None: VectorClock(vec)}))
    self.nc.all_engine_barrier()
    popped = self.nc._tile_sem_poison_stack.pop()
    assert popped is self._sem_poison
    self.nc.clear_and_free_semaphores(list(self.sems.allocated().values()))
    self.nc.all_engine_barrier()


tile.TileContext._drain_and_barrier = _patched_drain_and_barrier

FP = mybir.dt.float32
BF = mybir.dt.bfloat16
B, K, C, H, W = 8, 8, 256, 64, 64
HW = H * W
P = 128                 # SBUF partitions
NCT = C // P            # 2 c-tiles per core
MMN = 512               # moving free dim per matmul (= one PSUM bank of f32)
NMM = HW // MMN         # 8 matmuls per (k, c-tile)

_AF = mybir.ActivationFunctionType
_OP = mybir.AluOpType
_X = mybir.AxisListType.X


def build_nc() -> bass.Bass:
    nc = bass.Bass()
    IR = nc.declare_dram_parameter("IR", [K, C, HW], FP, isOutput=False)
    CR = nc.declare_dram_parameter("CR", [K, C, HW], FP, isOutput=False)
    CT = nc.declare_dram_parameter("CT", [C, HW], FP, isOutput=False)
    OUT = nc.declare_dram_parameter("OUT", [C, HW], BF, isOutput=True)

    with tile.TileContext(nc) as tc:
        with (
            tc.tile_pool(name="ctp", bufs=1) as ct_pool,
            tc.tile_pool(name="crp", bufs=5) as cr_pool,
            tc.tile_pool(name="irp", bufs=6) as ir_pool,
            tc.tile_pool(name="snk", bufs=1) as sink_pool,
            tc.tile_pool(name="sml", bufs=1) as small,
            tc.tile_pool(name="wkp", bufs=2) as wk_pool,
            tc.tile_pool(name="obp", bufs=2) as out_pool,
            tc.tile_pool(name="psp", bufs=1, space="PSUM") as psum_pool,
        ):
            acc = psum_pool.tile([P, HW], FP, name="acc")
            # Dead elementwise outputs of the phase-1 reductions; one sink
            # per engine so ACT and DVE never serialize on a WAW hazard.
            asink = sink_pool.tile([P, HW], BF, name="asink", tag="asink")
            vsink = sink_pool.tile([P, HW], BF, name="vsink", tag="vsink")

            # Diagonal ones mask, built once: mask[p, f] = (p == f).
            ones_t = small.tile([P, P], FP, name="ones_t")
            nc.vector.memset(ones_t[:], 1.0)
            mask = small.tile([P, P], FP, name="mask")
            nc.gpsimd.affine_select(
                mask[:],
                ones_t[:],
                pattern=[[-1, P]],
                compare_op=_OP.is_equal,
                fill=0.0,
                base=0,
                channel_multiplier=1,
            )

            # Content-target features stay resident in SBUF (reused by all k).
            ct_tiles = []
            for t in range(NCT):
                ctt = ct_pool.tile([P, HW], FP, name=f"ct{t}", tag=f"ct{t}")
                nc.sync.dma_start(out=ctt[:], in_=CT[t * P:(t + 1) * P, :])
                ct_tiles.append(ctt)

            # ---- Phase 1: dot[c,k] and sq[c,k] reductions over HW ----
            # ACT: ||CR||^2 via Square+accumulate.  DVE: dot via one fused
            # tensor_tensor_reduce (CT*CR, free-axis sum).  Both engines run
            # below the CR DMA pace, so this phase streams at HBM rate.
            dots = []
            sqs = []
            cr_gate = None
            for t in range(NCT):
                cs = slice(t * P, (t + 1) * P)
                dot = small.tile([P, K], FP, name=f"dot{t}", tag=f"dot{t}")
                sq = small.tile([P, K], FP, name=f"sq{t}", tag=f"sq{t}")
                for k in range(K):
                    crt = cr_pool.tile([P, HW], FP, name="crt", tag="cr")
                    nc.sync.dma_start(out=crt[:], in_=CR[k, cs, :])
                    if t * K + k == 6:  # CR tile #7 -> the IR-stream gate
                        cr_gate = crt
                    nc.scalar.activation(
                        out=asink[:], in_=crt[:], func=_AF.Square,
                        accum_out=sq[:, k:k + 1],
                    )
                    nc.vector.scalar_tensor_tensor(
                        out=vsink[:],
                        in0=ct_tiles[t][:],
                        scalar=1.0,
                        in1=crt[:],
                        op0=_OP.mult,
                        op1=_OP.mult,
                        accum_out=dot[:, k:k + 1],
                    )
                dots.append(dot)
                sqs.append(sq)

            # ---- Softmax weights (tiny [128, K] ops) ----
            # Both sqrts first, then both exps: exactly one Sqrt and one Exp
            # ACT table switch, overlapped with the IR DMA stream.
            norms = []
            for t in range(NCT):
                norm = small.tile([P, K], FP, name=f"norm{t}", tag=f"norm{t}")
                nc.scalar.sqrt(norm[:], sqs[t][:])
                norms.append(norm)
            ws = []
            for t in range(NCT):
                rnorm = small.tile([P, K], FP, name=f"rnorm{t}", tag=f"rnorm{t}")
                nc.vector.reciprocal(rnorm[:], norms[t][:])
                sim = small.tile([P, K], FP, name=f"sim{t}", tag=f"sim{t}")
                nc.vector.tensor_mul(sim[:], dots[t][:], rnorm[:])
                mx = small.tile([P, 1], FP, name=f"mx{t}", tag=f"mx{t}")
                nc.vector.reduce_max(mx[:], sim[:], axis=_X)
                nbias = small.tile([P, 1], FP, name=f"nb{t}", tag=f"nb{t}")
                nc.vector.tensor_scalar_mul(nbias[:], mx[:], -2.0)
                e = small.tile([P, K], FP, name=f"e{t}", tag=f"e{t}")
                nc.scalar.activation(
                    e[:], sim[:], func=_AF.Exp, bias=nbias[:, 0:1], scale=2.0
                )
                s = small.tile([P, 1], FP, name=f"s{t}", tag=f"s{t}")
                nc.vector.reduce_sum(s[:], e[:], axis=_X)
                rs = small.tile([P, 1], FP, name=f"rs{t}", tag=f"rs{t}")
                nc.vector.reciprocal(rs[:], s[:])
                w = small.tile([P, K], FP, name=f"w{t}", tag=f"w{t}")
                nc.vector.tensor_scalar_mul(w[:], e[:], rs[:, 0:1])
                ws.append(w)

            # ---- Phase 3: out[c,:] = sum_k IR[k,c,:] * w[c,k] ----
            # IR is cast f32->bf16 in flight (SWDGE), TensorE applies the
            # per-channel weight as a bf16 diagonal matmul into PSUM.
            QQ = HW // 4
            for t in range(NCT):
                cs = slice(t * P, (t + 1) * P)
                w = ws[t]
                for ki, k in enumerate(range(K)):
                    wm = wk_pool.tile([P, P], BF, name="wm", tag="wm")
                    nc.vector.tensor_scalar_mul(wm[:], mask[:], w[:, k:k + 1])
                    irt = ir_pool.tile([P, HW], BF, name="irt", tag="ir")
                    if k % 2 == 0:
                        if t == 0 and k == 0:
                            # Gate: scribble a CR-#10-derived value into the
                            # first IR tile so this SWDGE DMA (WAW on the
                            # tile) cannot start before CR #10 has landed;
                            # Q7 ring backpressure paces the rest.
                            nc.gpsimd.tensor_copy(
                                irt[0:1, 0:1], cr_gate[0:1, 0:1]
                            )
                        nc.gpsimd.dma_start(out=irt[:], in_=IR[k, cs, :])
                    else:
                        # Odd k: f32 on the post-CR-idle sync queue, reusing
                        # CR-pool slots (whose WARs naturally pace them to
                        # the CR tail), cast to bf16 on DVE/ACT.
                        irf = cr_pool.tile([P, HW], FP, name="irf", tag="cr")
                        nc.sync.dma_start(out=irf[:], in_=IR[k, cs, :])
                        if (k // 2) % 2 == 0:
                            nc.scalar.copy(irt[:], irf[:])
                        else:
                            # k=5,7 on DVE: keeps the last tile's cast off
                            # ACT, which still owes the PSUM quarter copies.
                            nc.vector.tensor_copy(irt[:], irf[:])
                    for j in range(NMM):
                        col = j * MMN
                        nc.tensor.matmul(
                            acc[:, col:col + MMN],
                            wm[:],
                            irt[:, col:col + MMN],
                            start=(ki == 0),
                            stop=(ki == K - 1),
                        )
                # PSUM -> SBUF on ACT (idle here), stream each quarter out
                # as soon as its last matmul lands.
                ob = out_pool.tile([P, HW], BF, name="ob", tag="ob")
                for q in range(4):
                    qs = slice(q * QQ, (q + 1) * QQ)
                    nc.scalar.copy(ob[:, qs], acc[:, qs])
                    # Scalar-queue (third HWDGE ring): output writes never
                    # contend with the sync queue's remaining IR loads.
                    nc.scalar.dma_start(out=OUT[cs, qs], in_=ob[:, qs])

    return nc


_NC_CACHE = None


def _get_nc() -> bass.Bass:
    global _NC_CACHE
    if _NC_CACHE is None:
        _NC_CACHE = build_nc()
    return _NC_CACHE


def run(inputs: dict, trace: bool = False):
    """Shard over B, run on 8 cores, gather. Returns (output, BassKernelResults)."""
    ir = np.ascontiguousarray(np.asarray(inputs["IR_features"], dtype=np.float32))
    cr = np.ascontiguousarray(np.asarray(inputs["CR_features"], dtype=np.float32))
    ct = np.ascontiguousarray(np.asarray(inputs["CT_feature"], dtype=np.float32))
    assert ir.shape == (B, K, C, H, W) and cr.shape == (B, K, C, H, W)
    assert ct.shape == (B, C, H, W)

    in_maps = [
        {
            "IR": ir[b].reshape(K, C, HW),
            "CR": cr[b].reshape(K, C, HW),
            "CT": ct[b].reshape(C, HW),
        }
        for b in range(B)
    ]
    res = run_bass_kernel_spmd(_get_nc(), in_maps, list(range(B)), trace=trace)
    out = np.stack([res.results[b]["OUT"] for b in range(B)])
    return out.reshape(B, C, H, W).astype(np.float32), res


def kernel(**inputs) -> np.ndarray:
    return run(inputs)[0]


# revision 25
# speedup vs baseline: 1.1804x; 1.1619x over previous
"""Trainium2 Bass kernel for the DFS-Mixer style-attention module.

Computation (per batch b):
    dot[k,c]  = sum_hw CT[c,hw] * CR[k,c,hw]
    norm[k,c] = sqrt(sum_hw CR[k,c,hw]^2)
    w[.,c]    = softmax_k(2 * dot[.,c] / norm[.,c])
    out[c,hw] = sum_k IR[k,c,hw] * w[k,c]

Sharding: data-parallel over batch B=8 across the 8 NeuronCores (one b per
core, no cross-core communication).

Per-core layout: C=256 tiled as 2 x 128 SBUF partitions, HW=4096 on the free
axis.  68 MB in / 4 MB out per core -> DMA-roofline kernel (~200 us at
358 GB/s).  Engine budget is kept under the DMA pace in every phase:

- Phase 1 (per CR tile, f32 exact): ACT does ||CR||^2 via Square with
  accumulate; DVE does the dot in a SINGLE fused scalar_tensor_tensor
  ((CT*1)*CR with free-axis sum accum_out).  Per c-tile: ACT ~30us,
  DVE ~35us vs 44.7us of CR DMA -> DMA-bound.
- Softmax over K=8 for both c-tiles back to back (one Sqrt and one Exp
  ACT table switch, hidden under the IR DMA stream).
- Phase 3: IR is cast f32->bf16 in flight by SWDGE (gpsimd) DMA; TensorE
  applies the per-channel weights as bf16 diagonal matmuls accumulating
  in PSUM (full rate, ~15us/c-tile vs 41us for fp32).  ACT (idle here)
  copies PSUM->SBUF per quarter; the output is written as bf16 (half the
  write traffic; quantization well within the error budget).
- Schedule: the CR stream owns the HWDGE sync queue SOLO first (SDMA
  round-robins active queues ~50/50 at packet granularity, so any
  concurrent IR traffic would delay the softmax gate 1:1).  The first
  SWDGE IR cast-DMA is gated on CR tile #10 via a tiny gpsimd read, so
  IR transfers only start overlapping the last few CR tiles; after that
  the IR stream runs on BOTH queues at once -- even k's as f32->bf16
  SWDGE cast-DMAs (gpsimd queue), odd k's as f32 HWDGE loads on the
  now-idle sync queue (reusing CR-pool slots) cast on DVE/ACT.
"""

import os
import sys

import numpy as np


def _import_concourse():
    try:
        import concourse.bass  # noqa: F401
    except ImportError:
        for p in ("/opt/trn_rl_repo", "/root/.axon_site/_ro/trn_rl_repo"):
            if os.path.isdir(p) and p not in sys.path:
                sys.path.insert(0, p)
        import concourse.bass  # noqa: F401


_import_concourse()

import concourse.bass as bass  # noqa: E402
import concourse.mybir as mybir  # noqa: E402
from concourse import tile  # noqa: E402
from concourse.bass_utils import run_bass_kernel_spmd  # noqa: E402
from concourse.vector_clock import ScopedClock, VectorClock  # noqa: E402


def _split_multiwait_bir(bir: bytes) -> bytes:
    """The neuronxcc walrus in this container encodes at most ONE sync-wait
    per instruction; Tile emits several.  Hoist extra waits onto same-engine
    NoOp instructions inserted immediately before the original instruction
    (engines execute in order, so waiting earlier on the same engine is
    semantically identical).  Sem *updates* are left untouched (a DMA's
    completion-inc cannot move to a sequencer NoOp)."""
    import json

    j = json.loads(bir)
    ctr = 0
    for f in j.get("functions", []):
        for bb in f.get("blocks", []):
            out_insts = []
            for ins in bb.get("instructions", []):
                si = ins.get("sync_info")
                waits = (si or {}).get("on_wait") or []
                if len(waits) > 1:
                    for w in waits[:-1]:
                        ctr += 1
                        nop = {
                            "engine": ins["engine"],
                            "ins": [],
                            "outs": [],
                            "name": f"waitsplit-{ctr}",
                            "opcode": "NoOp",
                            "sync_info": {"on_update": [], "on_wait": [w]},
                        }
                        if "debug" in ins:
                            nop["debug"] = ins["debug"]
                        out_insts.append(nop)
                    si["on_wait"] = [waits[-1]]
                out_insts.append(ins)
            bb["instructions"] = out_insts
    return json.dumps(j).encode()


_orig_to_json_bytes = bass.Bass.to_json_bytes


def _patched_to_json_bytes(self, *a, **kw):
    return _split_multiwait_bir(_orig_to_json_bytes(self, *a, **kw))


bass.Bass.to_json_bytes = _patched_to_json_bytes


def _patched_drain_and_barrier(self, tick_clock, wait_clock):
    # Stock TileContext exit emits one Drain waiting on every used semaphore,
    # which this walrus rejects ("Too many sync wait commands").  Emit one
    # Drain per semaphore instead.
    gc = tick_clock.global_clock
    n = len(gc)
    nonzero = [p for p in range(n) if gc[p] > 0] or [0]
    for p in nonzero:
        d = self.nc.sync.drain()
        vec = [gc[q] if q == p else 0 for q in range(n)]
        wait_clock.add_sem_waits(d.ins, ScopedClock({None: VectorClock(vec)}))
    self.nc.all_engine_barrier()
    popped = self.nc._tile_sem_poison_stack.pop()
    assert popped is self._sem_poison
    self.nc.clear_and_free_semaphores(list(self.sems.allocated().values()))
    self.nc.all_engine_barrier()


tile.TileContext._drain_and_barrier = _patched_drain_and_barrier

FP = mybir.dt.float32
BF = mybir.dt.bfloat16
B, K, C, H, W = 8, 8, 256, 64, 64
HW = H * W
P = 128                 # SBUF partitions
NCT = C // P            # 2 c-tiles per core
MMN = 512               # moving free dim per matmul (= one PSUM bank of f32)
NMM = HW // MMN         # 8 matmuls per (k, c-tile)

_AF = mybir.ActivationFunctionType
_OP = mybir.AluOpType
_X = mybir.AxisListType.X


def build_nc() -> bass.Bass:
    nc = bass.Bass()
    IR = nc.declare_dram_parameter("IR", [K, C, HW], FP, isOutput=False)
    CR = nc.declare_dram_parameter("CR", [K, C, HW], FP, isOutput=False)
    CT = nc.declare_dram_parameter("CT", [C, HW], FP, isOutput=False)
    OUT = nc.declare_dram_parameter("OUT", [C, HW], BF, isOutput=True)

    with tile.TileContext(nc) as tc:
        with (
            tc.tile_pool(name="ctp", bufs=1) as ct_pool,
            tc.tile_pool(name="crp", bufs=4) as cr_pool,
            tc.tile_pool(name="irp", bufs=6) as ir_pool,
            tc.tile_pool(name="snk", bufs=1) as sink_pool,
            tc.tile_pool(name="sml", bufs=1) as small,
            tc.tile_pool(name="wkp", bufs=2) as wk_pool,
            tc.tile_pool(name="obp", bufs=2) as out_pool,
            tc.tile_pool(name="psp", bufs=1, space="PSUM") as psum_pool,
        ):
            acc = psum_pool.tile([P, HW], FP, name="acc")
            # Dead elementwise outputs of the phase-1 reductions; one sink
            # per engine so ACT and DVE never serialize on a WAW hazard.
            asink = sink_pool.tile([P, HW], BF, name="asink", tag="asink")
            vsink = sink_pool.tile([P, HW], BF, name="vsink", tag="vsink")

            # Diagonal ones mask, built once: mask[p, f] = (p == f).
            ones_t = small.tile([P, P], FP, name="ones_t")
            nc.vector.memset(ones_t[:], 1.0)
            mask = small.tile([P, P], FP, name="mask")
            nc.gpsimd.affine_select(
                mask[:],
                ones_t[:],
                pattern=[[-1, P]],
                compare_op=_OP.is_equal,
                fill=0.0,
                base=0,
                channel_multiplier=1,
            )

            # Content-target features stay resident in SBUF (reused by all k).
            ct_tiles = []
            for t in range(NCT):
                ctt = ct_pool.tile([P, HW], FP, name=f"ct{t}", tag=f"ct{t}")
                nc.sync.dma_start(out=ctt[:], in_=CT[t * P:(t + 1) * P, :])
                ct_tiles.append(ctt)

            # ---- Phase 1: dot[c,k] and sq[c,k] reductions over HW ----
            # ACT: ||CR||^2 via Square+accumulate.  DVE: dot via one fused
            # tensor_tensor_reduce (CT*CR, free-axis sum).  Both engines run
            # below the CR DMA pace, so this phase streams at HBM rate.
            dots = []
            sqs = []
            cr_gate = None
            for t in range(NCT):
                cs = slice(t * P, (t + 1) * P)
                dot = small.tile([P, K], FP, name=f"dot{t}", tag=f"dot{t}")
                sq = small.tile([P, K], FP, name=f"sq{t}", tag=f"sq{t}")
                for k in range(K):
                    crt = cr_pool.tile([P, HW], FP, name="crt", tag="cr")
                    nc.sync.dma_start(out=crt[:], in_=CR[k, cs, :])
                    if t * K + k == 9:  # CR tile #10 -> the IR-stream gate
                        cr_gate = crt
                    nc.scalar.activation(
                        out=asink[:], in_=crt[:], func=_AF.Square,
                        accum_out=sq[:, k:k + 1],
                    )
                    nc.vector.scalar_tensor_tensor(
                        out=vsink[:],
                        in0=ct_tiles[t][:],
                        scalar=1.0,
                        in1=crt[:],
                        op0=_OP.mult,
                        op1=_OP.mult,
                        accum_out=dot[:, k:k + 1],
                    )
                dots.append(dot)
                sqs.append(sq)

            # ---- Softmax weights (tiny [128, K] ops) ----
            # Both sqrts first, then both exps: exactly one Sqrt and one Exp
            # ACT table switch, overlapped with the IR DMA stream.
            norms = []
            for t in range(NCT):
                norm = small.tile([P, K], FP, name=f"norm{t}", tag=f"norm{t}")
                nc.scalar.sqrt(norm[:], sqs[t][:])
                norms.append(norm)
            ws = []
            for t in range(NCT):
                rnorm = small.tile([P, K], FP, name=f"rnorm{t}", tag=f"rnorm{t}")
                nc.vector.reciprocal(rnorm[:], norms[t][:])
                sim = small.tile([P, K], FP, name=f"sim{t}", tag=f"sim{t}")
                nc.vector.tensor_mul(sim[:], dots[t][:], rnorm[:])
                mx = small.tile([P, 1], FP, name=f"mx{t}", tag=f"mx{t}")
                nc.vector.reduce_max(mx[:], sim[:], axis=_X)
                nbias = small.tile([P, 1], FP, name=f"nb{t}", tag=f"nb{t}")
                nc.vector.tensor_scalar_mul(nbias[:], mx[:], -2.0)
                e = small.tile([P, K], FP, name=f"e{t}", tag=f"e{t}")
                nc.scalar.activation(
                    e[:], sim[:], func=_AF.Exp, bias=nbias[:, 0:1], scale=2.0
                )
                s = small.tile([P, 1], FP, name=f"s{t}", tag=f"s{t}")
                nc.vector.reduce_sum(s[:], e[:], axis=_X)
                rs = small.tile([P, 1], FP, name=f"rs{t}", tag=f"rs{t}")
                nc.vector.reciprocal(rs[:], s[:])
                w = small.tile([P, K], FP, name=f"w{t}", tag=f"w{t}")
                nc.vector.tensor_scalar_mul(w[:], e[:], rs[:, 0:1])
                ws.append(w)

            # ---- Phase 3: out[c,:] = sum_k IR[k,c,:] * w[c,k] ----
            # IR is cast f32->bf16 in flight (SWDGE), TensorE applies the
            # per-channel weight as a bf16 diagonal matmul into PSUM.
            QQ = HW // 4
            for t in range(NCT):
                cs = slice(t * P, (t + 1) * P)
                w = ws[t]
                for ki, k in enumerate(range(K)):
                    wm = wk_pool.tile([P, P], BF, name="wm", tag="wm")
                    nc.vector.tensor_scalar_mul(wm[:], mask[:], w[:, k:k + 1])
                    irt = ir_pool.tile([P, HW], BF, name="irt", tag="ir")
                    if k % 2 == 0:
                        if t == 0 and k == 0:
                            # Gate: scribble a CR-#10-derived value into the
                            # first IR tile so this SWDGE DMA (WAW on the
                            # tile) cannot start before CR #10 has landed;
                            # Q7 ring backpressure paces the rest.
                            nc.gpsimd.tensor_copy(
                                irt[0:1, 0:1], cr_gate[0:1, 0:1]
                            )
                        nc.gpsimd.dma_start(out=irt[:], in_=IR[k, cs, :])
                    else:
                        # Odd k: f32 on the post-CR-idle sync queue, reusing
                        # CR-pool slots (whose WARs naturally pace them to
                        # the CR tail), cast to bf16 on DVE/ACT.
                        irf = cr_pool.tile([P, HW], FP, name="irf", tag="cr")
                        nc.sync.dma_start(out=irf[:], in_=IR[k, cs, :])
                        if (k // 2) % 2 == 0:
                            nc.vector.tensor_copy(irt[:], irf[:])
                        else:
                            nc.scalar.copy(irt[:], irf[:])
                    for j in range(NMM):
                        col = j * MMN
                        nc.tensor.matmul(
                            acc[:, col:col + MMN],
                            wm[:],
                            irt[:, col:col + MMN],
                            start=(ki == 0),
                            stop=(ki == K - 1),
                        )
                # PSUM -> SBUF on ACT (idle here), stream each quarter out
                # as soon as its last matmul lands.
                ob = out_pool.tile([P, HW], BF, name="ob", tag="ob")
                for q in range(4):
                    qs = slice(q * QQ, (q + 1) * QQ)
                    nc.scalar.copy(ob[:, qs], acc[:, qs])
                    nc.sync.dma_start(out=OUT[cs, qs], in_=ob[:, qs])

    return nc


_NC_CACHE = None


def _get_nc() -> bass.Bass:
    global _NC_CACHE
    if _NC_CACHE is None:
        _NC_CACHE = build_nc()
    return _NC_CACHE


def run(inputs: dict, trace: bool = False):
    """Shard over B, run on 8 cores, gather. Returns (output, BassKernelResults)."""
    ir = np.ascontiguousarray(np.asarray(inputs["IR_features"], dtype=np.float32))
    cr = np.ascontiguousarray(np.asarray(inputs["CR_features"], dtype=np.float32))
    ct = np.ascontiguousarray(np.asarray(inputs["CT_feature"], dtype=np.float32))
    assert ir.shape == (B, K, C, H, W) and cr.shape == (B, K, C, H, W)
    assert ct.shape == (B, C, H, W)

    in_maps = [
        {
            "IR": ir[b].reshape(K, C, HW),
            "CR": cr[b].reshape(K, C, HW),
            "CT": ct[b].reshape(C, HW),
        }
        for b in range(B)
    ]
    res = run_bass_kernel_spmd(_get_nc(), in_maps, list(range(B)), trace=trace)
    out = np.stack([res.results[b]["OUT"] for b in range(B)])
    return out.reshape(B, C, H, W).astype(np.float32), res


def kernel(**inputs) -> np.ndarray:
    return run(inputs)[0]
